# revision 2
# baseline (speedup 1.0000x reference)
"""Trainium2 Bass kernel for nn_CNNQNetwork (dueling CNN Q-network).

Sharding: pure data parallel — batch 4096 split as 512 samples on each of the
8 NeuronCores; all weights replicated.

Per-core layout: activations live in SBUF as [channel(partition), spatial, batch].
Convs are TensorE matmuls (one per kernel tap, accumulated in PSUM).
GroupNorm(1 group) per sample:
  - per-(c,b) partial sums via DVE tensor_reduce, squares via ScalarE
  - cross-channel reduction via tiny ones-matmuls (TensorE)
  - mean subtraction folded back into the conv PSUM as a K=1 matmul
  - gamma*rstd applied as a broadcast matmul + GpSimd multiply while casting
    features to bf16 for the MLP head matmuls
The dueling-head algebra (v + a - mean(a), biases) is folded into the second
linear layer's weights on the host.

Math notes used for exactness (verified against the reference):
  - relu(GroupNorm) with gamma>0, beta=0, conv bias=0 allows deferring the
    per-sample 1/std into the *feature* tensor only; intermediate blocks are
    scale invariant because GroupNorm(conv(r*u)) == GroupNorm(conv(u)).
  - per-channel gamma of h1/v1 is folded into the consuming conv weights.
"""

import numpy as np
import ml_dtypes

BF16 = ml_dtypes.bfloat16
B_TOTAL = 4096
NCORES = 8
BC = B_TOTAL // NCORES  # 512 samples per core
D = 128
EPS = 1e-5

# blocks: (name, src, kind, Hi, Wi, Ho, Wo)   kind 'h' = (1,2) kernel, 'v' = (2,1)
BLOCKS = [
    ("h1", "x2", "h", 4, 4, 4, 3),
    ("v1", "x3", "v", 4, 4, 3, 4),
    ("hh", "h1", "h", 4, 3, 4, 2),
    ("hv", "h1", "v", 4, 3, 3, 3),
    ("vh", "v1", "h", 3, 4, 3, 3),
    ("vv", "v1", "v", 3, 4, 2, 4),
]
S_OF = {n: ho * wo for (n, _, _, _, _, ho, wo) in BLOCKS}
NK = sum(S_OF.values())  # 58 K-slices of 128 for the head matmul

_cache = {}


def _build(loop_n=None):
    """Build the Bass program once. Returns (nc, meta)."""
    import concourse.bass as bass
    import concourse.tile as tile
    import concourse.mybir as mybir
    from concourse import bacc
    from concourse.masks import make_identity
    from contextlib import ExitStack

    dt = mybir.dt
    Alu = mybir.AluOpType
    Act = mybir.ActivationFunctionType

    from contextlib import nullcontext

    nc = bacc.Bacc(
        "TRN2",
        target_bir_lowering=False,
        debug=False,
        enable_asserts=False,
        num_devices=NCORES,
    )

    # ---- DRAM I/O ----
    x2_d = nc.dram_tensor("x2", [32, 16, BC], dt.bfloat16, kind="ExternalInput")
    x3_d = nc.dram_tensor("x3", [32, 16, BC], dt.bfloat16, kind="ExternalInput")
    cw1_d = nc.dram_tensor("cw1", [32, 256], dt.bfloat16, kind="ExternalInput")
    cw_d = nc.dram_tensor("cw", [128, 8 * 128], dt.bfloat16, kind="ExternalInput")
    hw_d = nc.dram_tensor("hw", [4, 128, NK * 128], dt.bfloat16, kind="ExternalInput")
    fw_d = nc.dram_tensor("fw", [128, 16], dt.bfloat16, kind="ExternalInput")
    hb_d = nc.dram_tensor("hb", [128, 4], dt.float32, kind="ExternalInput")
    b2_d = nc.dram_tensor("b2", [4, 1], dt.float32, kind="ExternalInput")
    gam_d = nc.dram_tensor("gam", [1, 6 * 128], dt.bfloat16, kind="ExternalInput")
    out_d = nc.dram_tensor("out", [BC, 4], dt.float32, kind="ExternalOutput")

    with tile.TileContext(nc) as tc, ExitStack() as ctx:
        singles = ctx.enter_context(tc.tile_pool(name="singles", bufs=1))
        rows = ctx.enter_context(tc.tile_pool(name="rows", bufs=4))
        sqp = ctx.enter_context(tc.tile_pool(name="sqp", bufs=6))
        stats = ctx.enter_context(tc.tile_pool(name="stats", bufs=2))
        uleafp = ctx.enter_context(tc.tile_pool(name="uleafp", bufs=2))

        # persistent SBUF tensors
        fw_sb = singles.tile([128, 16], dt.bfloat16, tag="fw", name="fw")
        hb_sb = singles.tile([128, 4], dt.float32, tag="hb", name="hb")
        b2_sb = singles.tile([4, 1], dt.float32, tag="b2", name="b2")
        gam_sb = singles.tile([1, 6 * 128], dt.bfloat16, tag="gam", name="gam")
        ident = singles.tile([128, 128], dt.float32, tag="ident", name="ident")
        ones_bf = singles.tile([128, 1], dt.bfloat16, tag="ones", name="ones")
        eps1 = singles.tile([1, 1], dt.float32, tag="eps1", name="eps1")
        nc.vector.memset(eps1[:], EPS)

        nc.sync.dma_start(fw_sb[:], fw_d[:])
        nc.sync.dma_start(hb_sb[:], hb_d[:])
        nc.sync.dma_start(b2_sb[:], b2_d[:])
        nc.sync.dma_start(gam_sb[:], gam_d[:])
        make_identity(nc, ident[:])
        nc.vector.memset(ones_bf[:], 1.0)

        # feature tiles (bf16, [c, s, b]) and the conv-chain activations
        feat = {}
        for name, _, _, _, _, ho, wo in BLOCKS:
            feat[name] = singles.tile([128, ho * wo, BC], dt.bfloat16, tag=f"f_{name}", name=f"f_{name}")
        u_keep = {
            "h1": singles.tile([128, 12, BC], dt.bfloat16, tag="u_h1", name="u_h1"),
            "v1": singles.tile([128, 12, BC], dt.bfloat16, tag="u_v1", name="u_v1"),
        }

        with (tc.For_i(0, loop_n, 1) if loop_n else nullcontext()):
            with (
                tc.tile_pool(name="convp", bufs=1) as convp,
                tc.tile_pool(name="zp", bufs=5, space="PSUM") as zp,
                tc.tile_pool(name="sp", bufs=2, space="PSUM") as sp,
                tc.tile_pool(name="gp", bufs=1, space="PSUM") as gp,
            ):
                x2_sb = convp.tile([32, 16, BC], dt.bfloat16, tag="x2", name="x2")
                x3_sb = convp.tile([32, 16, BC], dt.bfloat16, tag="x3", name="x3")
                cw1_sb = convp.tile([32, 256], dt.bfloat16, tag="cw1", name="cw1")
                cw_sb = convp.tile([128, 8 * 128], dt.bfloat16, tag="cw", name="cw")
                nc.sync.dma_start(x2_sb[:], x2_d[:])
                nc.sync.dma_start(x3_sb[:], x3_d[:])
                nc.sync.dma_start(cw1_sb[:], cw1_d[:])
                nc.sync.dma_start(cw_sb[:], cw_d[:])
                for bi, (name, src, kind, Hi, Wi, Ho, Wo) in enumerate(BLOCKS):
                    S = Ho * Wo
                    CS = 128 * S
                    first = src in ("x2", "x3")

                    if first:
                        sview = (x2_sb if src == "x2" else x3_sb)[:].rearrange(
                            "c (i j) b -> c i j b", i=Hi
                        )
                    else:
                        sview = u_keep[src][:].rearrange("c (i j) b -> c i j b", i=Hi)

                    def rhs_win(tap, b0, bn):
                        if kind == "h":
                            return sview[:, :, tap : tap + Wo, b0 : b0 + bn]
                        else:
                            return sview[:, tap : tap + Ho, :, b0 : b0 + bn]

                    u_dst = u_keep[name] if name in u_keep else uleafp.tile(
                        [128, S, BC], dt.bfloat16, tag="uleaf", name="uleaf"
                    )

                    zs = stats.tile([128, BC], dt.bfloat16, tag="zs", name="zs")
                    sqs = stats.tile([128, BC], dt.bfloat16, tag="sqs", name="sqs")
                    rrow = stats.tile([1, BC], dt.bfloat16, tag="rrow", name="rrow")

                    negl = rows.tile([1, 128], dt.bfloat16, tag="negl", name="negl")
                    nc.vector.memset(negl[:], -1.0 / CS)

                    for q in range(4):
                        q0 = q * 128
                        psS = sp.tile([1, 2, 128], dt.float32, tag="psS", name="psS")
                        zchunks = []
                        for chi in range(4):
                            b0 = q0 + chi * 32
                            zc = zp.tile([128, S, 32], dt.float32, tag="z", name="z")
                            zchunks.append((b0, zc))
                            if first:
                                lhsT = cw1_sb[:, bi * 128 : bi * 128 + 128]
                                nc.tensor.matmul(
                                    zc[:], lhsT, rhs_win(0, b0, 32), start=True, stop=True
                                )
                            else:
                                t0 = (bi - 2) * 2
                                for t in range(2):
                                    lhsT = cw_sb[:, (t0 + t) * 128 : (t0 + t + 1) * 128]
                                    nc.tensor.matmul(
                                        zc[:],
                                        lhsT,
                                        rhs_win(t, b0, 32),
                                        start=(t == 0),
                                        stop=(t == 1),
                                    )
                        with nc.allow_low_precision("bf16 groupnorm partial sums"):
                            for b0, zc in zchunks:
                                zt = zc[:].rearrange("c s b -> c b s")
                                nc.vector.tensor_reduce(
                                    zs[:, b0 : b0 + 32],
                                    zt,
                                    axis=mybir.AxisListType.X,
                                    op=Alu.add,
                                )
                                sq = sqp.tile([128, S, 32], dt.bfloat16, tag="sq", name="sq")
                                nc.scalar.square(sq[:], zc[:])
                                nc.vector.tensor_reduce(
                                    sqs[:, b0 : b0 + 32],
                                    sq[:].rearrange("c s b -> c b s"),
                                    axis=mybir.AxisListType.X,
                                    op=Alu.add,
                                )
                        # cross-channel sums: psS[0] = sum(zs), psS[1] = sum(sqs)
                        nc.tensor.matmul(
                            psS[:, 0, :], ones_bf[:], zs[:, q0 : q0 + 128],
                            start=True, stop=True,
                        )
                        nc.tensor.matmul(
                            psS[:, 1, :], ones_bf[:], sqs[:, q0 : q0 + 128],
                            start=True, stop=True,
                        )
                        # mu^2 = (sA/CS)^2 ; ve = sB/CS - mu^2 ; sd = sqrt(ve+eps); r = 1/sd
                        srow = rows.tile([1, 2, 128], dt.float32, tag="srow", name="srow")
                        nc.vector.tensor_copy(srow[:], psS[:])
                        mu2 = rows.tile([1, 128], dt.float32, tag="mu2", name="mu2")
                        nc.vector.scalar_tensor_tensor(
                            mu2[:], srow[:, 0, :], 1.0 / (CS * CS), srow[:, 0, :],
                            op0=Alu.mult, op1=Alu.mult,
                        )
                        ve = rows.tile([1, 128], dt.float32, tag="ve", name="ve")
                        nc.vector.scalar_tensor_tensor(
                            ve[:], srow[:, 1, :], 1.0 / CS, mu2[:],
                            op0=Alu.mult, op1=Alu.subtract,
                        )
                        sd = rows.tile([1, 128], dt.float32, tag="sd", name="sd")
                        nc.scalar.activation(
                            sd[:], ve[:], func=Act.Sqrt, bias=eps1[:], scale=1.0
                        )
                        rt = rows.tile([1, 128], dt.float32, tag="rt", name="rt")
                        nc.vector.reciprocal(rt[:], sd[:])
                        nc.vector.tensor_copy(rrow[:, q0 : q0 + 128], rt[:])
                        # bf16 row of sum(z) for the mean-subtract matmul
                        zrow = rows.tile([1, 128], dt.bfloat16, tag="zrow", name="zrow")
                        nc.vector.tensor_copy(zrow[:], srow[:, 0, :])
                        zq = rows.tile([1, S, 128], dt.bfloat16, tag="zq", name="zq")
                        nc.sync.dma_start(
                            zq[:], zrow[:, None, :].to_broadcast((1, S, 128))
                        )
                        # z -= mean  (K=1 matmul, lhsT = -1/CS)
                        for chi, (b0, zc) in enumerate(zchunks):
                            nc.tensor.matmul(
                                zc[:],
                                negl[:],
                                zq[:, :, chi * 32 : (chi + 1) * 32],
                                start=False,
                                stop=True,
                                skip_group_check=True,
                            )
                        # u = relu(z - mean)
                        for b0, zc in zchunks:
                            nc.scalar.activation(
                                u_dst[:, :, b0 : b0 + 32], zc[:], func=Act.Relu
                            )
                    # G[c,b] = gamma_c * r_b  (broadcast matmul), then feat = u * G
                    psG = gp.tile([128, BC], dt.float32, tag="psG", name="psG")
                    nc.tensor.matmul(
                        psG[:], gam_sb[:, bi * 128 : (bi + 1) * 128], rrow[:],
                        start=True, stop=True,
                    )
                    gsb = stats.tile([128, BC], dt.bfloat16, tag="gsb", name="gsb")
                    nc.scalar.copy(gsb[:], psG[:])
                    nc.gpsimd.tensor_tensor(
                        feat[name][:],
                        u_dst[:],
                        gsb[:, None, :].to_broadcast((128, S, BC)),
                        op=Alu.mult,
                    )

            # ---- heads ----
            with (
                tc.tile_pool(name="hwp", bufs=2) as hwp,
                tc.tile_pool(name="hidp", bufs=1) as hidp,
                tc.tile_pool(name="hp", bufs=2, space="PSUM") as hp,
                tc.tile_pool(name="fp", bufs=1, space="PSUM") as fp,
                tc.tile_pool(name="tp", bufs=2, space="PSUM") as tp,
            ):
                hids = []
                for mt in range(4):
                    hws = hwp.tile([128, NK * 128], dt.bfloat16, tag="hws", name="hws")
                    nc.sync.dma_start(hws[:], hw_d[mt])
                    psH = hp.tile([128, BC], dt.float32, tag="psH", name="psH")
                    k = 0
                    for name, _, _, _, _, ho, wo in BLOCKS:
                        for s in range(ho * wo):
                            nc.tensor.matmul(
                                psH[:],
                                hws[:, k * 128 : (k + 1) * 128],
                                feat[name][:, s, :],
                                start=(k == 0),
                                stop=(k == NK - 1),
                            )
                            k += 1
                    hid = hidp.tile([128, BC], dt.bfloat16, tag=f"hid{mt}", name=f"hid{mt}")
                    nc.scalar.activation(
                        hid[:], psH[:], func=Act.Relu, bias=hb_sb[:, mt : mt + 1], scale=1.0
                    )
                    hids.append(hid)
                psF = fp.tile([4, BC], dt.float32, tag="psF", name="psF")
                for mt in range(4):
                    nc.tensor.matmul(
                        psF[:],
                        fw_sb[:, mt * 4 : (mt + 1) * 4],
                        hids[mt][:],
                        start=(mt == 0),
                        stop=(mt == 3),
                    )
                finf = rows.tile([4, BC], dt.float32, tag="finf", name="finf")
                nc.scalar.activation(
                    finf[:], psF[:], func=Act.Identity, bias=b2_sb[:, 0:1], scale=1.0
                )
                osb = rows.tile([128, 4, 4], dt.float32, tag="osb", name="osb")
                for qq in range(4):
                    psT = tp.tile([128, 4], dt.float32, tag="psT", name="psT")
                    nc.tensor.transpose(
                        psT[:], finf[:, qq * 128 : (qq + 1) * 128], ident[0:4, 0:4]
                    )
                    nc.scalar.copy(osb[:, qq, :], psT[:])
                nc.sync.dma_start(out_d[:].rearrange("(q p) j -> p q j", p=128), osb[:])

    nc.compile()
    return nc


def _prep_weights(inp):
    """Host-side weight preprocessing shared by all cores."""
    f32 = np.float32
    for k in ("b_h1", "b_v1", "b_hh", "b_hv", "b_vh", "b_vv"):
        assert np.allclose(inp[k], 0.0), f"conv bias {k} must be zero"
    for k in ("gb_h1", "gb_v1", "gb_hh", "gb_hv", "gb_vh", "gb_vv"):
        assert np.allclose(inp[k], 0.0), f"groupnorm beta {k} must be zero"
    gammas = {n: np.asarray(inp[f"gw_{n}"], f32) for n in S_OF}
    for n, g in gammas.items():
        assert np.all(g > 0), f"gamma {n} must be positive"

    # first-level conv lhsT (taps stacked into K=32)
    w_h1 = np.asarray(inp["w_h1"], f32)
    w_v1 = np.asarray(inp["w_v1"], f32)
    cw1 = np.zeros((32, 256), f32)
    cw1[0:16, 0:128] = w_h1[:, :, 0, 0].T
    cw1[16:32, 0:128] = w_h1[:, :, 0, 1].T
    cw1[0:16, 128:256] = w_v1[:, :, 0, 0].T
    cw1[16:32, 128:256] = w_v1[:, :, 1, 0].T

    # second-level conv lhsT with parent's gamma folded in
    cw = np.zeros((128, 8 * 128), f32)
    second = [
        ("hh", "w_hh", "h1", "h"),
        ("hv", "w_hv", "h1", "v"),
        ("vh", "w_vh", "v1", "h"),
        ("vv", "w_vv", "v1", "v"),
    ]
    for idx, (name, wk, parent, kind) in enumerate(second):
        w = np.asarray(inp[wk], f32)
        g = gammas[parent]
        for t in range(2):
            tap = w[:, :, 0, t] if kind == "h" else w[:, :, t, 0]
            cw[:, (2 * idx + t) * 128 : (2 * idx + t + 1) * 128] = (tap * g[None, :]).T

    # head weights: W1c = [vw1; aw1] (512, 7424), re-tiled per (mtile, block, s)
    W1c = np.concatenate(
        [np.asarray(inp["vw1"], f32), np.asarray(inp["aw1"], f32)], axis=0
    )
    cols = []
    off = 0
    for name, _, _, _, _, ho, wo in BLOCKS:
        S = ho * wo
        Wb = W1c[:, off : off + 128 * S].reshape(512, 128, S)
        off += 128 * S
        for s in range(S):
            cols.append(Wb[:, :, s])
    K = np.stack(cols, 0)  # (58, 512, 128c)
    hw = np.empty((4, 128, NK * 128), f32)
    for mt in range(4):
        hw[mt] = K[:, mt * 128 : (mt + 1) * 128, :].transpose(2, 0, 1).reshape(128, -1)

    # final layer with dueling algebra folded in
    vw2 = np.asarray(inp["vw2"], f32)  # (1, 256)
    aw2 = np.asarray(inp["aw2"], f32)  # (4, 256)
    W2c = np.zeros((4, 512), f32)
    W2c[:, 0:256] = vw2[0][None, :]
    W2c[:, 256:512] = aw2 - aw2.mean(axis=0, keepdims=True)
    W2cT = W2c.T  # (512, 4)
    fw = np.zeros((128, 16), f32)
    for kt in range(4):
        fw[:, kt * 4 : (kt + 1) * 4] = W2cT[kt * 128 : (kt + 1) * 128, :]
    b2 = (
        np.asarray(inp["vb2"], f32)[0]
        + np.asarray(inp["ab2"], f32)
        - np.asarray(inp["ab2"], f32).mean()
    ).reshape(4, 1)

    hb = np.concatenate(
        [np.asarray(inp["vb1"], f32), np.asarray(inp["ab1"], f32)]
    ).reshape(4, 128).T.copy()  # [128, 4], column mt

    gam = np.zeros((1, 6 * 128), f32)
    for bi, (name, _, _, _, _, _, _) in enumerate(BLOCKS):
        gam[0, bi * 128 : (bi + 1) * 128] = gammas[name]

    return {
        "cw1": cw1.astype(BF16),
        "cw": cw.astype(BF16),
        "hw": hw.astype(BF16),
        "fw": fw.astype(BF16),
        "hb": hb.astype(np.float32),
        "b2": b2.astype(np.float32),
        "gam": gam.astype(BF16),
    }


def _prep_x(xs):
    """Per-core input prep: build the tap-stacked, [c,s,b] bf16 arrays."""
    f32 = np.float32
    n = xs.shape[0]
    x2 = np.zeros((n, 32, 4, 4), f32)
    x2[:, 0:16] = xs
    x2[:, 16:32, :, 0:3] = xs[:, :, :, 1:4]
    x3 = np.zeros((n, 32, 4, 4), f32)
    x3[:, 0:16] = xs
    x3[:, 16:32, 0:3, :] = xs[:, :, 1:4, :]
    x2 = x2.transpose(1, 2, 3, 0).reshape(32, 16, n)
    x3 = x3.transpose(1, 2, 3, 0).reshape(32, 16, n)
    return x2.astype(BF16), x3.astype(BF16)


def _get_nc():
    if "nc" not in _cache:
        _cache["nc"] = _build()
    return _cache["nc"]


def _core_in_map(inputs, c, _wcache={}):
    key = id(inputs)
    if _wcache.get("key") != key:
        _wcache["key"] = key
        _wcache["w"] = _prep_weights(inputs)
        _wcache["x"] = np.asarray(inputs["x"], np.float32)
    x = _wcache["x"]
    xs = x[c * BC : (c + 1) * BC]
    x2, x3 = _prep_x(xs)
    m = dict(_wcache["w"])
    m["x2"] = x2
    m["x3"] = x3
    return m


def _unpack_out(out):
    return np.asarray(out, np.float32)


def kernel(**inputs) -> np.ndarray:
    from concourse.bass_utils import run_bass_kernel_spmd

    nc = _get_nc()
    in_maps = [_core_in_map(inputs, c) for c in range(NCORES)]
    res = run_bass_kernel_spmd(nc, in_maps, core_ids=list(range(NCORES)))
    out = np.concatenate([_unpack_out(r["out"]) for r in res.results], axis=0)
    return out.astype(np.float32)



# revision 10
# speedup vs baseline: 1.4233x; 1.4233x over previous
"""Trainium2 Bass kernel for nn_CNNQNetwork (dueling CNN Q-network).

Sharding: pure data parallel — batch 4096 split as 512 samples on each of the
8 NeuronCores; all weights replicated.

v2 design (vs baseline): fewer, larger ops everywhere; GroupNorm stats
restructured so the PE stays dense and the DVE does two big reduces per
(block, 128-sample chunk) instead of many small strided ones.

Per-core layout: activations in SBUF as [channel(partition), spatial, batch].
Per (block, q=128-sample chunk): conv taps accumulate into a PSUM tile
[128, S, 128] (3 banks); matmuls are split at 2KB PSUM bank boundaries
(rectangular (i,j) sub-windows). GroupNorm(1 group):
  - zs[c,b] = sum_s z via DVE reduce (PSUM f32)
  - sq = z^2 via ScalarE -> [c,b,s] bf16, sqs[c,b] via dense DVE reduce
  - cross-channel sums via ones-matmul -> [1,512] rows in PSUM
  - row math: mu (ScalarE copy*1/CS), mu2 (ScalarE square*1/CS),
    ve (DVE STT), sd=sqrt(ve+eps) (ScalarE), r=1/sd (DVE recip_approx_fast)
  - mean subtraction folded into the conv PSUM as a K=1 matmul (lhsT=-1,
    rhs=mu row broadcast)
  - u = relu(z - mu) (ScalarE evict), feat = u * (r bcast) (GpSimd)
Per-channel gammas are folded into consuming conv weights and the head W1
host-side; per-sample 1/std appears only via feat (scale invariance of
GroupNorm makes intermediate u unnormalized-safe). The dueling-head algebra
is folded into the second linear layer host-side.
"""

import numpy as np
import ml_dtypes

BF16 = ml_dtypes.bfloat16
B_TOTAL = 4096
NCORES = 8
BC = B_TOTAL // NCORES  # 512 samples per core
D = 128
EPS = 1e-5

# blocks: (name, src, kind, Hi, Wi, Ho, Wo)   kind 'h' = (1,2) kernel, 'v' = (2,1)
BLOCKS = [
    ("h1", "x", "h", 4, 4, 4, 3),
    ("v1", "x", "v", 4, 4, 3, 4),
    ("hh", "h1", "h", 4, 3, 4, 2),
    ("hv", "h1", "v", 4, 3, 3, 3),
    ("vh", "v1", "h", 3, 4, 3, 3),
    ("vv", "v1", "v", 3, 4, 2, 4),
]
S_OF = {n: ho * wo for (n, _, _, _, _, ho, wo) in BLOCKS}
NK = sum(S_OF.values())  # 58 K-slices of 128 for the head matmul

_cache = {}
DEBUG_DUMP = False


def _regions(Ho, Wo):
    """Rectangular (i0, ni, j0, nj) output sub-windows whose flattened
    (s, b128) PSUM columns stay within one 2KB bank (4 s-positions)."""
    regs = []
    if Wo == 4:
        for i in range(Ho):
            regs.append((i, 1, 0, 4))
    elif Wo == 2:
        for i in range(0, Ho, 2):
            regs.append((i, 2, 0, 2))
    else:  # Wo == 3
        for i in range(Ho):
            j = 0
            while j < Wo:
                s = i * Wo + j
                jn = min(Wo - j, 4 - (s % 4))
                regs.append((i, 1, j, jn))
                j += jn
    return regs


def _build():
    """Build the Bass program once. Returns nc."""
    import concourse.bass as bass
    import concourse.tile as tile
    import concourse.mybir as mybir
    from concourse import bacc
    from concourse.masks import make_identity
    from contextlib import ExitStack

    dt = mybir.dt
    Alu = mybir.AluOpType
    Act = mybir.ActivationFunctionType

    nc = bacc.Bacc(
        "TRN2",
        target_bir_lowering=False,
        debug=False,
        enable_asserts=False,
        num_devices=NCORES,
    )

    # ---- DRAM I/O ----
    xt_d = nc.dram_tensor("xt", [64, 16, BC], dt.bfloat16, kind="ExternalInput")
    cw1_d = nc.dram_tensor("cw1", [64, 128], dt.bfloat16, kind="ExternalInput")
    cw_d = nc.dram_tensor("cw", [128, 8 * 128], dt.bfloat16, kind="ExternalInput")
    hw_d = nc.dram_tensor("hw", [4, 128, NK * 128], dt.bfloat16, kind="ExternalInput")
    fw_d = nc.dram_tensor("fw", [128, 16], dt.bfloat16, kind="ExternalInput")
    hb_d = nc.dram_tensor("hb", [128, 4], dt.float32, kind="ExternalInput")
    b2_d = nc.dram_tensor("b2", [4, 1], dt.float32, kind="ExternalInput")
    out_d = nc.dram_tensor("out", [BC, 4], dt.float32, kind="ExternalOutput")
    if DEBUG_DUMP:
        dbg_u = {
            n: nc.dram_tensor(f"dbg_u_{n}", [128, 12, BC], dt.bfloat16, kind="ExternalOutput")
            for n in ("h1", "v1")
        }
        dbg_f = {
            n: nc.dram_tensor(f"dbg_f_{n}", [128, S_OF[n], BC], dt.bfloat16, kind="ExternalOutput")
            for n in S_OF
        }

    with tile.TileContext(nc) as tc, ExitStack() as ctx:
        singles = ctx.enter_context(tc.tile_pool(name="singles", bufs=1))
        rows = ctx.enter_context(tc.tile_pool(name="rows", bufs=2))
        sqp = ctx.enter_context(tc.tile_pool(name="sqp", bufs=3))
        statp = ctx.enter_context(tc.tile_pool(name="statp", bufs=2))
        uleafp = ctx.enter_context(tc.tile_pool(name="uleafp", bufs=2))
        gsbp = ctx.enter_context(tc.tile_pool(name="gsbp", bufs=2))

        # persistent SBUF tensors
        fw_sb = singles.tile([128, 16], dt.bfloat16, tag="fw", name="fw")
        hb_sb = singles.tile([128, 4], dt.float32, tag="hb", name="hb")
        b2_sb = singles.tile([4, 1], dt.float32, tag="b2", name="b2")
        ident = singles.tile([128, 128], dt.float32, tag="ident", name="ident")
        ones_col = singles.tile([128, 1], dt.bfloat16, tag="ones_col", name="ones_col")
        posones = singles.tile([1, 128], dt.bfloat16, tag="posones", name="posones")
        negones = singles.tile([1, 128], dt.bfloat16, tag="negones", name="negones")
        eps1 = singles.tile([1, 1], dt.float32, tag="eps1", name="eps1")
        nc.vector.memset(eps1[:], EPS)
        nc.vector.memset(ones_col[:], 1.0)
        nc.vector.memset(posones[:], 1.0)
        nc.vector.memset(negones[:], -1.0)

        nc.sync.dma_start(fw_sb[:], fw_d[:])
        nc.sync.dma_start(hb_sb[:], hb_d[:])
        nc.sync.dma_start(b2_sb[:], b2_d[:])
        make_identity(nc, ident[:])

        # activations (bf16, [c, s, b])
        feat = {}
        for name, _, _, _, _, ho, wo in BLOCKS:
            feat[name] = singles.tile(
                [128, ho * wo, BC], dt.bfloat16, tag=f"f_{name}", name=f"f_{name}"
            )
        u_keep = {
            "h1": singles.tile([128, 12, BC], dt.bfloat16, tag="u_h1", name="u_h1"),
            "v1": singles.tile([128, 12, BC], dt.bfloat16, tag="u_v1", name="u_v1"),
        }

        with (
            tc.tile_pool(name="convp", bufs=1) as convp,
            tc.tile_pool(name="zp", bufs=2, space="PSUM") as zp,
            tc.tile_pool(name="sp", bufs=1, space="PSUM") as sp,
            tc.tile_pool(name="gp", bufs=1, space="PSUM") as gp,
        ):
            xt_sb = convp.tile([64, 16, BC], dt.bfloat16, tag="xt", name="xt")
            cw1_sb = convp.tile([64, 128], dt.bfloat16, tag="cw1", name="cw1")
            cw_sb = convp.tile([128, 8 * 128], dt.bfloat16, tag="cw", name="cw")
            nc.sync.dma_start(xt_sb[:], xt_d[:])
            nc.sync.dma_start(cw1_sb[:], cw1_d[:])
            nc.sync.dma_start(cw_sb[:], cw_d[:])

            for bi, (name, src, kind, Hi, Wi, Ho, Wo) in enumerate(BLOCKS):
                S = Ho * Wo
                CS = 128 * S
                first = src == "x"
                regs = _regions(Ho, Wo)

                if first:
                    base = 0 if kind == "h" else 32
                    sview = xt_sb[base : base + 32].rearrange(
                        "c (i j) b -> c i j b", i=Hi
                    )
                else:
                    sview = u_keep[src][:].rearrange("c (i j) b -> c i j b", i=Hi)

                def rhs_win(t, i0, ni, j0, nj, b0):
                    if kind == "h":
                        return sview[:, i0 : i0 + ni, j0 + t : j0 + t + nj, b0 : b0 + 128]
                    else:
                        return sview[:, i0 + t : i0 + t + ni, j0 : j0 + nj, b0 : b0 + 128]

                u_dst = u_keep[name] if name in u_keep else uleafp.tile(
                    [128, 9, BC], dt.bfloat16, tag="uleaf", name=f"u_{name}"
                )

                zs = statp.tile([128, BC], dt.bfloat16, tag="zs", name="zs")
                sqs = statp.tile([128, BC], dt.bfloat16, tag="sqs", name="sqs")

                # cross-channel sum rows accumulate per chunk: zs sums on
                # partition 0, sqs sums on partition 32 of one PSUM bank
                psZQ = sp.tile([64, BC], dt.float32, tag="zq", name="psZQ")
                mu_row = rows.tile([1, BC], dt.bfloat16, tag="mu_row", name="mu_row")

                for q in range(4):
                    b0 = q * 128
                    zt = zp.tile([128, 12, 128], dt.float32, tag="z", name=f"z_{name}{q}")
                    seen_banks = set()
                    for (i0, ni, j0, nj) in regs:
                        s0 = i0 * Wo + j0
                        n = ni * nj
                        dst = zt[:, s0 : s0 + n, :]
                        # start=True clears has_written for the WHOLE 2KB bank,
                        # so only the first matmul touching a bank may set it.
                        bank = s0 // 4
                        bank_first = bank not in seen_banks
                        seen_banks.add(bank)
                        if first:
                            lhsT = cw1_sb[base : base + 32, :]
                            nc.tensor.matmul(
                                dst, lhsT, rhs_win(0, i0, ni, j0, nj, b0),
                                start=bank_first, stop=True,
                                skip_group_check=not bank_first,
                            )
                        else:
                            t0 = (bi - 2) * 2
                            for t in range(2):
                                lhsT = cw_sb[:, (t0 + t) * 128 : (t0 + t + 1) * 128]
                                nc.tensor.matmul(
                                    dst, lhsT, rhs_win(t, i0, ni, j0, nj, b0),
                                    start=(t == 0 and bank_first), stop=(t == 1),
                                    skip_group_check=not bank_first,
                                )
                    # stats partials for this chunk
                    with nc.allow_low_precision("bf16 groupnorm partial sums"):
                        nc.vector.tensor_reduce(
                            zs[:, b0 : b0 + 128],
                            zt[:, :S, :].rearrange("c s b -> c b s"),
                            axis=mybir.AxisListType.X,
                            op=Alu.add,
                        )
                        sq = sqp.tile([128, 128, 12], dt.bfloat16, tag="sq", name="sq")
                        nc.scalar.activation(
                            sq[:, :, :S],
                            zt[:, :S, :].rearrange("c s b -> c b s"),
                            func=Act.Square,
                        )
                        nc.vector.tensor_reduce(
                            sqs[:, b0 : b0 + 128],
                            sq[:, :, :S],
                            axis=mybir.AxisListType.X,
                            op=Alu.add,
                        )
                    nc.tensor.matmul(
                        psZQ[0:1, b0 : b0 + 128], ones_col[:], zs[:, b0 : b0 + 128],
                        start=True, stop=True,
                    )
                    nc.tensor.matmul(
                        psZQ[32:33, b0 : b0 + 128], ones_col[:], sqs[:, b0 : b0 + 128],
                        start=True, stop=True,
                    )
                    nc.scalar.activation(
                        mu_row[:, b0 : b0 + 128], psZQ[0:1, b0 : b0 + 128],
                        func=Act.Copy, scale=1.0 / CS,
                    )
                    # mean subtraction (K=1 matmul into existing PSUM) + relu evict
                    murow_q = mu_row[:, b0 : b0 + 128]
                    for (i0, ni, j0, nj) in regs:
                        s0 = i0 * Wo + j0
                        n = ni * nj
                        nc.tensor.matmul(
                            zt[:, s0 : s0 + n, :],
                            negones[:],
                            murow_q[:, None, :].to_broadcast((1, n, 128)),
                            start=False, stop=True, skip_group_check=True,
                        )
                    nc.scalar.activation(
                        u_dst[:, :S, b0 : b0 + 128], zt[:, :S, :], func=Act.Relu
                    )

                # per-block row math for r = 1/std
                mu2 = rows.tile([1, BC], dt.float32, tag="mu2", name="mu2")
                nc.scalar.activation(
                    mu2[:], psZQ[0:1, :], func=Act.Square, scale=1.0 / CS
                )
                ve = rows.tile([1, BC], dt.float32, tag="ve", name="ve")
                nc.vector.scalar_tensor_tensor(
                    ve[:], psZQ[32:33, :], 1.0 / CS, mu2[:],
                    op0=Alu.mult, op1=Alu.subtract,
                )
                sd = rows.tile([1, BC], dt.float32, tag="sd", name="sd")
                nc.scalar.activation(sd[:], ve[:], func=Act.Sqrt, bias=eps1[:], scale=1.0)
                r0 = rows.tile([1, BC], dt.float32, tag="r0", name="r0")
                nc.vector.reciprocal_approx_fast(out=r0[:], in_=sd[:])
                rb = rows.tile([1, BC], dt.bfloat16, tag="rb", name="rb")
                nc.scalar.activation(rb[:], r0[:], func=Act.Copy)

                # G = r broadcast to 128 partitions
                psG = gp.tile([128, BC], dt.float32, tag="g", name="psG")
                nc.tensor.matmul(psG[:], posones[:], rb[:], start=True, stop=True)
                gsb = gsbp.tile([128, BC], dt.bfloat16, tag="gsb", name="gsb")
                nc.vector.tensor_copy(gsb[:], psG[:])

                # feat = u * r
                nc.gpsimd.tensor_tensor(
                    feat[name][:],
                    u_dst[:, :S, :],
                    gsb[:, None, :].to_broadcast((128, S, BC)),
                    op=Alu.mult,
                )
                if DEBUG_DUMP:
                    if name in dbg_u:
                        nc.sync.dma_start(dbg_u[name][:], u_dst[:])
                    nc.sync.dma_start(dbg_f[name][:], feat[name][:])

        # ---- heads ----
        with (
            tc.tile_pool(name="hwp", bufs=2) as hwp,
            tc.tile_pool(name="hidp", bufs=1) as hidp,
            tc.tile_pool(name="hp", bufs=2, space="PSUM") as hp,
            tc.tile_pool(name="fp", bufs=1, space="PSUM") as fp,
            tc.tile_pool(name="tp", bufs=2, space="PSUM") as tp,
        ):
            hids = []
            for mt in range(4):
                hws = hwp.tile([128, NK * 128], dt.bfloat16, tag="hws", name="hws")
                nc.sync.dma_start(hws[:], hw_d[mt])
                psH = hp.tile([128, BC], dt.float32, tag="psH", name="psH")
                k = 0
                for name, _, _, _, _, ho, wo in BLOCKS:
                    for s in range(ho * wo):
                        nc.tensor.matmul(
                            psH[:],
                            hws[:, k * 128 : (k + 1) * 128],
                            feat[name][:, s, :],
                            start=(k == 0),
                            stop=(k == NK - 1),
                        )
                        k += 1
                hid = hidp.tile([128, BC], dt.bfloat16, tag=f"hid{mt}", name=f"hid{mt}")
                nc.scalar.activation(
                    hid[:], psH[:], func=Act.Relu, bias=hb_sb[:, mt : mt + 1], scale=1.0
                )
                hids.append(hid)
            psF = fp.tile([4, BC], dt.float32, tag="psF", name="psF")
            for mt in range(4):
                nc.tensor.matmul(
                    psF[:],
                    fw_sb[:, mt * 4 : (mt + 1) * 4],
                    hids[mt][:],
                    start=(mt == 0),
                    stop=(mt == 3),
                )
            finf = rows.tile([4, BC], dt.float32, tag="finf", name="finf")
            nc.scalar.activation(
                finf[:], psF[:], func=Act.Identity, bias=b2_sb[:, 0:1], scale=1.0
            )
            osb = rows.tile([128, 4, 4], dt.float32, tag="osb", name="osb")
            for qq in range(4):
                psT = tp.tile([128, 4], dt.float32, tag="psT", name="psT")
                nc.tensor.transpose(
                    psT[:], finf[:, qq * 128 : (qq + 1) * 128], ident[0:4, 0:4]
                )
                nc.scalar.copy(osb[:, qq, :], psT[:])
            nc.sync.dma_start(out_d[:].rearrange("(q p) j -> p q j", p=128), osb[:])

    nc.compile()
    return nc


def _prep_weights(inp):
    """Host-side weight preprocessing shared by all cores."""
    f32 = np.float32
    for k in ("b_h1", "b_v1", "b_hh", "b_hv", "b_vh", "b_vv"):
        assert np.allclose(inp[k], 0.0), f"conv bias {k} must be zero"
    for k in ("gb_h1", "gb_v1", "gb_hh", "gb_hv", "gb_vh", "gb_vv"):
        assert np.allclose(inp[k], 0.0), f"groupnorm beta {k} must be zero"
    gammas = {n: np.asarray(inp[f"gw_{n}"], f32) for n in S_OF}
    for n, g in gammas.items():
        assert np.all(g > 0), f"gamma {n} must be positive"

    # first-level conv lhsT: [64, 128] — rows 0:32 h1 taps, 32:64 v1 taps
    w_h1 = np.asarray(inp["w_h1"], f32)
    w_v1 = np.asarray(inp["w_v1"], f32)
    cw1 = np.zeros((64, 128), f32)
    cw1[0:16] = w_h1[:, :, 0, 0].T
    cw1[16:32] = w_h1[:, :, 0, 1].T
    cw1[32:48] = w_v1[:, :, 0, 0].T
    cw1[48:64] = w_v1[:, :, 1, 0].T

    # second-level conv lhsT with parent's gamma folded in
    cw = np.zeros((128, 8 * 128), f32)
    second = [
        ("hh", "w_hh", "h1", "h"),
        ("hv", "w_hv", "h1", "v"),
        ("vh", "w_vh", "v1", "h"),
        ("vv", "w_vv", "v1", "v"),
    ]
    for idx, (name, wk, parent, kind) in enumerate(second):
        w = np.asarray(inp[wk], f32)
        g = gammas[parent]
        for t in range(2):
            tap = w[:, :, 0, t] if kind == "h" else w[:, :, t, 0]
            cw[:, (2 * idx + t) * 128 : (2 * idx + t + 1) * 128] = (tap * g[None, :]).T

    # head weights: W1c = [vw1; aw1] (512, 7424), block gammas folded in,
    # re-tiled per (mtile, block, s)
    W1c = np.concatenate(
        [np.asarray(inp["vw1"], f32), np.asarray(inp["aw1"], f32)], axis=0
    )
    cols = []
    off = 0
    for name, _, _, _, _, ho, wo in BLOCKS:
        S = ho * wo
        Wb = W1c[:, off : off + 128 * S].reshape(512, 128, S)
        Wb = Wb * gammas[name][None, :, None]
        off += 128 * S
        for s in range(S):
            cols.append(Wb[:, :, s])
    K = np.stack(cols, 0)  # (58, 512, 128c)
    hw = np.empty((4, 128, NK * 128), f32)
    for mt in range(4):
        hw[mt] = K[:, mt * 128 : (mt + 1) * 128, :].transpose(2, 0, 1).reshape(128, -1)

    # final layer with dueling algebra folded in
    vw2 = np.asarray(inp["vw2"], f32)  # (1, 256)
    aw2 = np.asarray(inp["aw2"], f32)  # (4, 256)
    W2c = np.zeros((4, 512), f32)
    W2c[:, 0:256] = vw2[0][None, :]
    W2c[:, 256:512] = aw2 - aw2.mean(axis=0, keepdims=True)
    W2cT = W2c.T  # (512, 4)
    fw = np.zeros((128, 16), f32)
    for kt in range(4):
        fw[:, kt * 4 : (kt + 1) * 4] = W2cT[kt * 128 : (kt + 1) * 128, :]
    b2 = (
        np.asarray(inp["vb2"], f32)[0]
        + np.asarray(inp["ab2"], f32)
        - np.asarray(inp["ab2"], f32).mean()
    ).reshape(4, 1)

    hb = np.concatenate(
        [np.asarray(inp["vb1"], f32), np.asarray(inp["ab1"], f32)]
    ).reshape(4, 128).T.copy()  # [128, 4], column mt

    return {
        "cw1": cw1.astype(BF16),
        "cw": cw.astype(BF16),
        "hw": hw.astype(BF16),
        "fw": fw.astype(BF16),
        "hb": hb.astype(np.float32),
        "b2": b2.astype(np.float32),
    }


def _prep_x(xs):
    """Per-core input prep: [64, 16, n] tap-stacked bf16 (h taps 0:32, v 32:64)."""
    f32 = np.float32
    n = xs.shape[0]
    xt = np.zeros((n, 64, 4, 4), f32)
    xt[:, 0:16] = xs
    xt[:, 16:32, :, 0:3] = xs[:, :, :, 1:4]
    xt[:, 32:48] = xs
    xt[:, 48:64, 0:3, :] = xs[:, :, 1:4, :]
    xt = xt.transpose(1, 2, 3, 0).reshape(64, 16, n)
    return xt.astype(BF16)


def _get_nc():
    if "nc" not in _cache:
        _cache["nc"] = _build()
    return _cache["nc"]


def _core_in_map(inputs, c, _wcache={}):
    key = id(inputs)
    if _wcache.get("key") != key:
        _wcache["key"] = key
        _wcache["w"] = _prep_weights(inputs)
        _wcache["x"] = np.asarray(inputs["x"], np.float32)
    x = _wcache["x"]
    m = dict(_wcache["w"])
    m["xt"] = _prep_x(x[c * BC : (c + 1) * BC])
    return m


def _unpack_out(out):
    return np.asarray(out, np.float32)


def kernel(**inputs) -> np.ndarray:
    from concourse.bass_utils import run_bass_kernel_spmd

    nc = _get_nc()
    in_maps = [_core_in_map(inputs, c) for c in range(NCORES)]
    res = run_bass_kernel_spmd(nc, in_maps, core_ids=list(range(NCORES)))
    out = np.concatenate([_unpack_out(r["out"]) for r in res.results], axis=0)
    return out.astype(np.float32)
